# revision 59
# baseline (speedup 1.0000x reference)
"""Trainium2 Bass kernel for BiomarkerConditionedAttention.

Sharding: pure data-parallel over batch B=16 across 8 cores (2 batches/core).

Layout strategy (per core; "T" = feature-on-partitions):
  - host stages x both token-major and channel-major, weights pre-transposed,
    all matmul operands cast to bf16 (PSUM accumulation stays fp32).
  - branch1: q/k channel-major; v token-major, widened with a per-head
    ones-column so attn@v also emits the softmax denominator row.
  - branch2 cross-attention via rank-1 algebra: scores=(q_ca@wk)@x.T,
    ctx_h=wv_h@(attn@x)+bv_h - avoids full k/v projections of x.
  - branch3 grid_sample as dense matmul with on-device trilinear hat weights.
  - gate: per-token part contracts only standard_out; conditioned/dba slabs
    are per-batch row terms added via a K=1 ones-row matmul.

Scheduling: the PE HAM clock-gate only un-throttles (1.2->2.4 GHz) under
sustained high-utilization matmul activity and re-throttles after ~3.4us of
idling.  The serial bio-chain (tiny matmuls + DVE/ACT hops) therefore must
never own the PE timeline: its stages are interleaved as closures pumped
between qkv/v-projection iterations (window 1) and between self-attention
head-pairs (window 2), keeping the PE dense from the first weight DMA to
the last gate matmul.  Softmax denominators are collected per batch and
inverted with two batched reciprocals instead of 24 serial [1,513] ones.
"""

import os
import sys

sys.path.insert(0, "/opt/trn_rl_repo")

import ml_dtypes
import numpy as np

import concourse.bass as bass
import concourse.mybir as mybir
import concourse.tile as tile
from concourse import bacc, bass_utils

F32 = mybir.dt.float32
BF = mybir.dt.bfloat16
NPBF = ml_dtypes.bfloat16
AF = mybir.ActivationFunctionType
ALU = mybir.AluOpType
AX = mybir.AxisListType

B, N, C, H, M, G = 16, 513, 768, 12, 5, 8
HD = C // H  # 64
ISC = float(1.0 / np.sqrt(HD))
NCORES = 8
BPC = B // NCORES  # 2
KC = C // 128  # 6
TOKC = [103, 103, 103, 102, 102]
TOK0 = [0, 103, 206, 309, 411]
VW = H * (HD + 1)  # 780
NQ = [(0, 512), (512, 1)]  # psum-bank-aligned chunks of 513
N768 = [(0, 512), (512, 256)]
NVW = [(0, 512), (512, 268)]


def build(nc: bass.Bass):
    dram = {}

    def din(name, shape, dt=F32):
        dram[name] = nc.dram_tensor(name, list(shape), dt, kind="ExternalInput")

    din("xT", (BPC, C, N), BF)
    din("xn", (BPC, N, C), BF)
    din("bioT", (C, BPC), BF)
    din("confb", (128, BPC))
    din("confT", (BPC, 1))
    din("offsT", (1, 3, BPC * M))
    din("bcoordsT", (1, 3, BPC * M))
    din("w_qkT", (C, 2 * C), BF)
    din("w_vT", (C, VW), BF)
    din("w_caqT", (C, C), BF)
    din("w_cak", (C, C), BF)
    din("w_cavT", (C, C), BF)
    din("w_bioqT", (C, C), BF)
    din("w_caoT", (C, C), BF)
    din("w_dbaqT", (C, C), BF)
    din("w_dbakvT", (C, 2 * C), BF)
    din("w_dbaoT", (C, C), BF)
    din("w_spT", (C, C), BF)
    din("w_g1T", (C, C), BF)
    din("w_g2T", (C, C), BF)
    din("w_g3T", (C, C), BF)
    din("w_pT", (C, C), BF)
    din("b_qk", (2 * C,))
    din("b_vw", (1, VW), BF)
    din("b_bioq", (C,))
    din("b_caq", (C,))
    din("b_cav", (C,))
    din("b_cao", (C,))
    din("b_dbaq", (C,))
    din("b_dbav", (C,))
    din("b_dbao", (C,))
    din("b_sp", (C,))
    din("b_g", (BPC, C), BF)
    din("selr", (BPC, BPC, 513), BF)
    din("b_p", (C,))
    din("ident", (128, 128), BF)
    din("ngrid", (512, 3))
    din("splitT", (2, 128), BF)
    din("hsplit", (128, 2), BF)

    out = nc.dram_tensor("outT", [BPC, C, N], F32, kind="ExternalOutput")
    dram["densX"] = nc.dram_tensor("densX", [BPC, H, 513], F32, kind="Internal")

    with tile.TileContext(nc) as tc:
        emit(nc, tc, dram, out)
    nc.compile()
    return dram, out


def emit(nc, tc, dram, out):
    sync, vec, act, gp, pe = nc.sync, nc.vector, nc.scalar, nc.gpsimd, nc.tensor
    mmr = pe.matmul

    def wload(pool, wname, cols=C, colofs=0, name=None, dt=BF, tag="wbig"):
        t = pool.tile([128, KC, cols], dt, tag=tag, name=name or f"w_{wname}")
        src = dram[wname].ap()
        if cols != src.shape[1] or colofs:
            src = src[:, colofs : colofs + cols]
        sync.dma_start(out=t, in_=src.rearrange("(k p) m -> p k m", p=128))
        return t

    with tc.tile_pool(name="consts", bufs=1) as consts, tc.tile_pool(
        name="persist", bufs=1
    ) as persist, tc.tile_pool(name="wpool", bufs=4) as wpool, tc.tile_pool(
        name="smallA", bufs=1
    ) as smA:
        import contextlib as _ctxlib

        _x_stack = _ctxlib.ExitStack()
        xpool = _x_stack.enter_context(tc.tile_pool(name="xpool", bufs=1))
        # ------- critical-path DMAs first: x (chan-major) + qk weights -------
        xT = xpool.tile([128, BPC, KC, N], BF)
        xsrc = dram["xT"].ap().rearrange("b (k p) n -> p b k n", p=128)
        wqsrc = dram["w_qkT"].ap().rearrange("(k p) m -> p k m", p=128)
        wqk0 = wpool.tile([128, KC, C], BF, tag="wbig", name="wqk0")
        wqk1 = wpool.tile([128, KC, C], BF, tag="wbig", name="wqk1")
        sync.dma_start(out=wqk0[:, :, 0:384], in_=wqsrc[:, :, 0:384])
        for k in range(KC):
            sync.dma_start(out=xT[:, 0, k], in_=xsrc[:, 0, k])
        sync.dma_start(out=wqk0[:, :, 384:768], in_=wqsrc[:, :, 384:768])
        for half in range(2):
            sync.dma_start(
                out=wqk1[:, :, 384 * half : 384 * (half + 1)],
                in_=wqsrc[:, :, C + 384 * half : C + 384 * (half + 1)],
            )
        for k in range(KC):
            sync.dma_start(out=xT[:, 1, k], in_=xsrc[:, 1, k])
        xg = xpool.tile([128, BPC, 4, C], BF)
        xa = xpool.tile([1, BPC, C], BF)
        for b in range(BPC):
            sync.dma_start(out=xa[:, b], in_=dram["xn"].ap()[b, 0:1, :])
            sync.dma_start(
                out=xg[:, b],
                in_=dram["xn"].ap()[b, 1:513, :].rearrange("(t p) c -> p t c", p=128),
            )

        # ------------- constants & tiny inputs (gp queue) -------------
        ident = consts.tile([128, 128], BF)
        gp.dma_start(out=ident, in_=dram["ident"].ap())
        splitT = consts.tile([2, 128], BF)
        gp.dma_start(out=splitT, in_=dram["splitT"].ap())
        hsplit = consts.tile([128, 2], BF)
        gp.dma_start(out=hsplit, in_=dram["hsplit"].ap())
        ngrid = consts.tile([128, 4, 3], F32)
        gp.dma_start(
            out=ngrid, in_=dram["ngrid"].ap().rearrange("(t p) d -> p t d", p=128)
        )
        bioT = consts.tile([128, KC, BPC], BF)
        gp.dma_start(
            out=bioT, in_=dram["bioT"].ap().rearrange("(k p) b -> p k b", p=128)
        )
        confb = consts.tile([128, BPC], F32)
        gp.dma_start(out=confb, in_=dram["confb"].ap())
        confT = consts.tile([BPC, 1], F32)
        gp.dma_start(out=confT, in_=dram["confT"].ap())

        def bias_col(name, nchunk):
            t = consts.tile([128, nchunk], F32, name=f"bc_{name}")
            gp.dma_start(out=t, in_=dram[name].ap().rearrange("(k p) -> p k", p=128))
            return t

        bqk = bias_col("b_qk", 12)
        bbioq = bias_col("b_bioq", KC)
        bcaq = bias_col("b_caq", KC)
        bcav = bias_col("b_cav", KC)
        bcao = bias_col("b_cao", KC)
        bdbaq = bias_col("b_dbaq", KC)
        bdbav = bias_col("b_dbav", KC)
        bdbao = bias_col("b_dbao", KC)
        bsp = bias_col("b_sp", KC)
        bp = bias_col("b_p", KC)
        bg = consts.tile([BPC, C], BF)
        gp.dma_start(out=bg, in_=dram["b_g"].ap())
        bvw = consts.tile([128, VW], BF)
        gp.dma_start(out=bvw, in_=dram["b_vw"].ap().broadcast_to((128, VW)))

        # persistent activations
        qkT = persist.tile([128, BPC, 18, N], BF)
        vn = persist.tile([128, BPC, 5, VW + 63], BF)
        pixb = persist.tile([128, 3, BPC * M], F32)
        wtri = persist.tile([128, 4, BPC * M], BF)
        sampT = persist.tile([128, KC, BPC * M], BF)
        sampPT = persist.tile([128, KC, BPC * M], BF)
        kvdba = persist.tile([128, 12, BPC * M], BF)
        bioqT = persist.tile([128, KC, BPC], BF)
        qcaT = persist.tile([128, KC, BPC], BF)
        qdbaT = persist.tile([128, KC, BPC], BF)
        qkc = persist.tile([128, KC, H, BPC], BF)
        xbarT = persist.tile([128, KC, BPC * H], BF)
        ctxcaT = persist.tile([128, KC, BPC], BF)
        condT = persist.tile([128, KC, BPC], BF)
        ctxdT = persist.tile([128, KC, BPC], BF)
        dbaT = persist.tile([128, KC, BPC], BF)
        bioc = persist.tile([128, KC, BPC], F32)
        samp = persist.tile([M, BPC, C], BF)

        # one-time zeroing of the static pad regions (K=128/M=128 tricks)
        vec.memset(qkT[64:128, :, 6:12, :], 0.0)
        vec.memset(qkT[0:64, :, 12:18, :], 0.0)
        vec.memset(vn, 0.0)

        # --- trilinear hat weights (DVE/ACT/gp only; fills the DMA wait) ---
        offsT = smA.tile([1, 3, BPC * M], F32)
        gp.dma_start(out=offsT, in_=dram["offsT"].ap())
        baseT = smA.tile([1, 3, BPC * M], F32)
        gp.dma_start(out=baseT, in_=dram["bcoordsT"].ap())
        pixT = smA.tile([1, 3, BPC * M], F32)
        vec.tensor_tensor(out=pixT, in0=offsT, in1=baseT, op=ALU.add)
        vec.tensor_scalar(pixT, pixT, -1.0, 1.0, ALU.max, ALU.min)
        vec.tensor_scalar(pixT, pixT, 3.5, 3.5, ALU.mult, ALU.add)
        for d in range(3):
            gp.partition_broadcast(pixb[:, d], pixT[:, d, :])
        wd = smA.tile([128, 3, BPC * M], F32)
        wxy = smA.tile([128, BPC * M], F32)
        for t in range(4):
            for d in range(3):
                act.activation(wd[:, d], pixb[:, d], AF.Abs, bias=ngrid[:, t, d : d + 1])
                act.activation(wd[:, d], wd[:, d], AF.Relu, bias=1.0, scale=-1.0)
            vec.tensor_tensor(out=wxy, in0=wd[:, 0], in1=wd[:, 1], op=ALU.mult)
            vec.tensor_tensor(out=wtri[:, t], in0=wxy, in1=wd[:, 2], op=ALU.mult)

        # =================== window 1: qkv/v + interleaved bio ===============
        with tc.tile_pool(name="ppA", bufs=2, space="PSUM") as ppA:

            def proj_small(w, rhs_tile, bias, o, pool, tag, bufs=2):
                nf = rhs_tile.shape[-1]
                for mo in range(KC):
                    ps = pool.tile([128, nf], F32, tag=tag, bufs=bufs, name="ps_proj")
                    for k in range(KC):
                        mmr(
                            ps,
                            w[:, k, mo * 128 : (mo + 1) * 128],
                            rhs_tile[:, k, :],
                            start=(k == 0),
                            stop=(k == KC - 1),
                        )
                    if bias is None:
                        vec.tensor_copy(out=o[:, mo], in_=ps)
                    else:
                        vec.tensor_scalar_add(o[:, mo], ps, bias[:, mo : mo + 1])

            # ---- bio-chain stages (window-1 portion), pumped between iters ----
            wref = {}

            def st_samp(b):
                def f():
                    ps = ppA.tile([M, C], F32, tag="psm")
                    for lo, sz in N768:
                        for t in range(4):
                            mmr(
                                ps[:, lo : lo + sz],
                                wtri[:, t, b * M : (b + 1) * M],
                                xg[:, b, t, lo : lo + sz],
                                start=(t == 0),
                                stop=(t == 3),
                            )
                    vec.tensor_copy(out=samp[:, b], in_=ps)
                return f

            def st_sampT(b):
                def f():
                    for k in range(KC):
                        pst = ppA.tile([128, M], BF, tag="psm")
                        pe.transpose(
                            pst, samp[:, b, k * 128 : (k + 1) * 128], ident[:M, :M]
                        )
                        vec.tensor_copy(
                            out=sampT[:, k, b * M : (b + 1) * M], in_=pst
                        )
                return f

            def st_load(wname, key, tag="wmed", **kw):
                def f():
                    wref[key] = wload(wpool, wname, name=key, tag=tag, **kw)
                return f

            def st_proj(key, rhs, bias, o):
                def f():
                    proj_small(wref[key], rhs, bias, o, ppA, "psm")
                return f

            def st_qkc(h0, h1):
                def f():
                    for h in range(h0, h1):
                        po, kk = 64 * (h % 2), h // 2
                        for mo in range(KC):
                            ps = ppA.tile([128, BPC], F32, tag="psm")
                            mmr(
                                ps,
                                wref["wcak"][po : po + 64, kk, mo * 128 : (mo + 1) * 128],
                                qcaT[po : po + 64, kk, :],
                                start=True,
                                stop=True,
                            )
                            vec.tensor_copy(out=qkc[:, mo, h], in_=ps)
                return f

            def st_ca_scores(b):
                def f():
                    ps = ppA.tile([H, 513], F32, tag="psm")
                    for lo, sz in NQ:
                        for k in range(KC):
                            mmr(
                                ps[:, lo : lo + sz],
                                qkc[:, k, :, b],
                                xT[:, b, k, lo : lo + sz],
                                start=(k == 0),
                                stop=(k == KC - 1),
                            )
                    attn = smA.tile([H, 513], BF, tag="attnca", bufs=2, name="attnca")
                    den = smA.tile([H, 2], F32, tag="denca", bufs=2, name="denca")
                    act.activation(
                        attn[:, 0:512], ps[:, 0:512], AF.Exp, scale=ISC,
                        accum_out=den[:, 0:1],
                    )
                    act.activation(
                        attn[:, 512:513], ps[:, 512:513], AF.Exp, scale=ISC,
                        accum_out=den[:, 1:2],
                    )
                    vec.tensor_tensor(
                        out=den[:, 0:1], in0=den[:, 0:1], in1=den[:, 1:2], op=ALU.add
                    )
                    vec.reciprocal(den[:, 0:1], den[:, 0:1])
                    vec.tensor_scalar_mul(attn, attn, den[:, 0:1])
                    wref[("attn", b)] = attn
                return f

            def st_ca_xbar(b):
                def f():
                    attn = wref[("attn", b)]
                    attnT = smA.tile([128, 5, H], BF, tag="attnTca", name="attnTca")
                    pst0 = ppA.tile([1, H], BF, tag="psm")
                    pe.transpose(pst0, attn[:, 0:1], ident[:H, :H])
                    vec.tensor_copy(out=attnT[0:1, 0, :], in_=pst0)
                    for t in range(4):
                        pst = ppA.tile([128, H], BF, tag="psm")
                        pe.transpose(
                            pst, attn[:, 1 + 128 * t : 1 + 128 * (t + 1)],
                            ident[:H, :H],
                        )
                        vec.tensor_copy(out=attnT[:, 1 + t, :], in_=pst)
                    psx = ppA.tile([H, C], F32, tag="psm")
                    for lo, sz in N768:
                        mmr(
                            psx[:, lo : lo + sz],
                            attnT[0:1, 0, :],
                            xa[:, b, lo : lo + sz],
                            start=True,
                            stop=False,
                        )
                        for t in range(4):
                            mmr(
                                psx[:, lo : lo + sz],
                                attnT[:, 1 + t, :],
                                xg[:, b, t, lo : lo + sz],
                                start=False,
                                stop=(t == 3),
                            )
                    xbar = smA.tile([H, C], BF, tag="xbarca", name="xbarca")
                    vec.tensor_copy(out=xbar, in_=psx)
                    for k in range(KC):
                        pst = ppA.tile([128, H], BF, tag="psm")
                        pe.transpose(
                            pst, xbar[:, k * 128 : (k + 1) * 128], ident[:H, :H]
                        )
                        vec.tensor_copy(out=xbarT[:, k, b * H : (b + 1) * H], in_=pst)
                return f

            # schedule: (emission slot -> stage) across 24 qkv iterations
            sched = {
                3: st_load("w_bioqT", "wbioq"),
                4: st_load("w_dbaqT", "wdbaq"),
                10: st_samp(0),
                11: st_samp(1),
                12: st_sampT(0),
                13: st_sampT(1),
                14: st_proj("wbioq", bioT, bbioq, bioqT),
                16: st_proj("wdbaq", bioT, bdbaq, qdbaT),
                17: st_load("w_caqT", "wcaq"),
                20: st_proj("wcaq", bioqT, bcaq, qcaT),
                21: st_load("w_cak", "wcak"),
                23: st_load("w_spT", "wsp"),
            }

            it = 0

            def tick():
                nonlocal it
                it += 1
                if it in sched:
                    sched[it]()

            # --- qkv projections (24 iterations, batch-major) ---
            for b in range(BPC):
                for m in range(12):
                    w = wqk0 if m < KC else wqk1
                    mo = m % KC
                    ps = ppA.tile([128, 513], F32, tag="pbig")
                    for lo, sz in NQ:
                        for k in range(KC):
                            mmr(
                                ps[:, lo : lo + sz],
                                w[:, k, mo * 128 : (mo + 1) * 128],
                                xT[:, b, k, lo : lo + sz],
                                start=(k == 0),
                                stop=(k == KC - 1),
                            )
                    if m < KC:
                        vec.tensor_scalar_add(qkT[:, b, m, :], ps, bqk[:, m : m + 1])
                    else:
                        # k: write evens-padded (rows 0:64) and odds-padded
                        # (rows 64:128) blocks; the other half stays zero.
                        vec.tensor_scalar_add(
                            qkT[0:64, b, m, :], ps[0:64], bqk[0:64, m : m + 1]
                        )
                        vec.tensor_scalar_add(
                            qkT[64:128, b, m + KC, :],
                            ps[64:128],
                            bqk[64:128, m : m + 1],
                        )
                    tick()

            # v weights (ring slots free after wbioq/wdbaq projections)
            wva = wpool.tile([128, KC, 512], BF, tag="wbig", name="wva")
            sync.dma_start(
                out=wva,
                in_=dram["w_vT"].ap()[:, 0:512].rearrange("(k p) m -> p k m", p=128),
            )
            wvb = wpool.tile([128, KC, 268], BF, tag="wbig", name="wvb")
            sync.dma_start(
                out=wvb,
                in_=dram["w_vT"].ap()[:, 512:780].rearrange("(k p) m -> p k m", p=128),
            )

            vsched = {
                2: st_qkc(0, 6),
                3: st_qkc(6, 12),
                4: st_ca_scores(0),
                5: st_ca_xbar(0),
                6: st_ca_scores(1),
                7: st_ca_xbar(1),
                8: st_proj("wsp", sampT, bsp, sampPT),
                9: st_load("w_dbakvT", "wdkv0", cols=C, colofs=0),
            }
            vit = 0
            for b in range(BPC):
                for t in range(5):
                    o0, cs = TOK0[t], TOKC[t]
                    ps = ppA.tile([128, VW], F32, tag="pbig")
                    for (lo, sz), wv in ((NVW[0], wva), (NVW[1], wvb)):
                        for k in range(KC):
                            mmr(
                                ps[:cs, lo : lo + sz],
                                xT[:, b, k, o0 : o0 + cs],
                                wv[:, k, :sz],
                                start=(k == 0),
                                stop=(k == KC - 1),
                            )
                    vec.tensor_tensor(
                        out=vn[:cs, b, t, 0:VW], in0=ps[:cs], in1=bvw[:cs], op=ALU.add
                    )
                    vit += 1
                    if vit in vsched:
                        vsched[vit]()

        _x_stack.close()

        # =========== window 2: self-attention + bio-chain drain ==============
        import contextlib

        with tc.tile_pool(name="soPool", bufs=1) as soP:
            _sa_stack = contextlib.ExitStack()
            smB = _sa_stack.enter_context(tc.tile_pool(name="smallB", bufs=1))
            ppB = _sa_stack.enter_context(
                tc.tile_pool(name="ppB", bufs=2, space="PSUM")
            )
            soT = soP.tile([128, BPC, KC, N], BF)

            def drain_proj(key, rhs, bias, o):
                def f():
                    proj_small(wref[key], rhs, bias, o, ppB, "psm2", bufs=2)
                return f

            def d_kv(part):
                def f():
                    w = wref["wdkv0"] if part == 0 else wref["wdkv1"]
                    for mo in range(KC):
                        m = part * KC + mo
                        ps = ppB.tile([128, BPC * M], F32, tag="psm2", bufs=2)
                        for k in range(KC):
                            mmr(
                                ps,
                                w[:, k, mo * 128 : (mo + 1) * 128],
                                sampPT[:, k, :],
                                start=(k == 0),
                                stop=(k == KC - 1),
                            )
                        if part == 0:
                            vec.tensor_copy(out=kvdba[:, m], in_=ps)
                        else:
                            vec.tensor_scalar_add(
                                kvdba[:, m], ps, bdbav[:, mo : mo + 1]
                            )
                return f

            def d_ctxca():
                def f():
                    for h in range(H):
                        po, kk = 64 * (h % 2), h // 2
                        ps = ppB.tile([64, BPC], F32, tag="psm2", bufs=2)
                        for k in range(KC):
                            mmr(
                                ps,
                                wref["wcav"][:, k, 64 * h : 64 * (h + 1)],
                                xbarT[:, k, h : 2 * H : H],
                                start=(k == 0),
                                stop=(k == KC - 1),
                            )
                        vec.tensor_scalar_add(
                            ctxcaT[po : po + 64, kk, :],
                            ps,
                            bcav[po : po + 64, kk : kk + 1],
                        )
                return f

            def d_dba_a():
                def f():
                    prod = smA.tile([128, BPC * M], BF, tag="prod", name="prod")
                    edba = smA.tile([2, KC, BPC * M], BF, tag="edba", name="edba")
                    ddba = smA.tile([2, KC, BPC], F32, tag="ddba", name="ddba")
                    for kk in range(KC):
                        vec.tensor_tensor(
                            out=prod.rearrange("p (b m) -> p b m", b=BPC),
                            in0=kvdba[:, kk, :].rearrange("p (b m) -> p b m", b=BPC),
                            in1=qdbaT[:, kk, :].unsqueeze(2).broadcast_to(
                                (128, BPC, M)
                            ),
                            op=ALU.mult,
                        )
                        pd = ppB.tile([2, BPC * M], F32, tag="psm2", bufs=2)
                        mmr(pd, hsplit, prod, start=True, stop=True)
                        act.activation(edba[:, kk, :], pd, AF.Exp, scale=ISC)
                    vec.tensor_reduce(
                        ddba,
                        edba.rearrange("p k (b m) -> p k b m", b=BPC),
                        axis=AX.X,
                        op=ALU.add,
                    )
                    vec.reciprocal(ddba, ddba)
                    for kk in range(KC):
                        for b in range(BPC):
                            vec.tensor_scalar_mul(
                                edba[:, kk, b * M : (b + 1) * M],
                                edba[:, kk, b * M : (b + 1) * M],
                                ddba[:, kk, b : b + 1],
                            )
                    wref["edba"] = edba
                    wref["prod"] = prod
                return f

            def d_dba_b():
                def f():
                    edba, prod = wref["edba"], wref["prod"]
                    for kk in range(KC):
                        psb = ppB.tile([128, BPC * M], F32, tag="psm2", bufs=2)
                        mmr(psb, splitT, edba[:, kk, :], start=True, stop=True)
                        vec.tensor_tensor(
                            out=prod, in0=kvdba[:, KC + kk, :], in1=psb, op=ALU.mult
                        )
                        with nc.allow_low_precision(reason="5-term reduce"):
                            vec.tensor_reduce(
                                ctxdT[:, kk, :],
                                prod.rearrange("p (b m) -> p b m", b=BPC),
                                axis=AX.X,
                                op=ALU.add,
                            )
                return f

            def d_bioc():
                def f():
                    for b in range(BPC):
                        vec.tensor_scalar_mul(
                            bioc[:, :, b : b + 1],
                            dbaT[:, :, b : b + 1],
                            confb[:, b : b + 1],
                        )
                    vec.tensor_tensor(out=bioc, in0=bioc, in1=condT, op=ALU.add)
                    for b in range(BPC):
                        vec.tensor_scalar(
                            bioc[:, :, b : b + 1],
                            bioc[:, :, b : b + 1],
                            confb[:, b : b + 1],
                            0.5,
                            ALU.mult,
                            ALU.mult,
                        )
                return f

            def d_row():
                def f():
                    rowf = smA.tile([BPC, C], F32, tag="rowf", name="rowf")
                    for lo, sz in N768:
                        p2 = ppB.tile([BPC, sz], F32, tag="psm2", bufs=2)
                        for k in range(KC):
                            mmr(
                                p2,
                                condT[:, k, :],
                                wref["wg2"][:, k, lo : lo + sz],
                                start=(k == 0),
                                stop=(k == KC - 1),
                            )
                        p3 = ppB.tile([BPC, sz], F32, tag="psm2", bufs=2)
                        for k in range(KC):
                            mmr(
                                p3,
                                dbaT[:, k, :],
                                wref["wg3"][:, k, lo : lo + sz],
                                start=(k == 0),
                                stop=(k == KC - 1),
                            )
                        vec.tensor_scalar_mul(rowf[:, lo : lo + sz], p3, confT)
                        vec.tensor_tensor(
                            out=rowf[:, lo : lo + sz],
                            in0=rowf[:, lo : lo + sz],
                            in1=p2,
                            op=ALU.add,
                        )
                    rowt = smA.tile([BPC, C], BF, tag="rowt", name="rowt")
                    vec.tensor_tensor(out=rowt, in0=rowf, in1=bg, op=ALU.add)
                    wref["rowt"] = rowt
                return f

            def d_selr():
                def f():
                    selr = smA.tile([BPC, BPC, 513], BF, tag="selr", name="selr")
                    sync.dma_start(out=selr, in_=dram["selr"].ap())
                    wref["selr"] = selr
                return f

            drains = [
                st_load("w_g2T", "wg2", tag="wbig"),
                st_load("w_g3T", "wg3", tag="wbig"),
                st_load("w_dbakvT", "wdkv1", cols=C, colofs=C),
                d_kv(0),
                st_load("w_cavT", "wcav"),
                d_kv(1),
                st_load("w_caoT", "wcao"),
                d_ctxca(),
                st_load("w_dbaoT", "wdbao"),
                drain_proj("wcao", ctxcaT, bcao, condT),
                d_dba_a(),
                d_dba_b(),
                drain_proj("wdbao", ctxdT, bdbao, dbaT),
                d_bioc(),
                st_load("w_g1T", "wg1", tag="wbig"),
                st_load("w_pT", "wp", tag="wbig"),
                d_row(),
                d_selr(),
            ]

            def gate_mo(b, mo, pp=None, gbufs=2):
                def f():
                    po_ = pp or ppB
                    wg1, wp, rowt, selr = (
                        wref["wg1"], wref["wp"], wref["rowt"], wref["selr"],
                    )
                    fusedT = wref[("fusedT", b)]
                    ps = po_.tile([128, 512], F32, tag="pgate", bufs=gbufs)
                    psx = po_.tile([128, 8], F32, tag="psm2", bufs=2)
                    for k in range(KC):
                        mmr(
                            ps,
                            wg1[:, k, mo * 128 : (mo + 1) * 128],
                            soT[:, b, k, 0:512],
                            start=(k == 0),
                            stop=False,
                        )
                    mmr(
                        ps,
                        rowt[:, mo * 128 : (mo + 1) * 128],
                        selr[:, b, 0:512],
                        start=False,
                        stop=True,
                    )
                    for k in range(KC):
                        mmr(
                            psx[:, 0:1],
                            wg1[:, k, mo * 128 : (mo + 1) * 128],
                            soT[:, b, k, 512:513],
                            start=(k == 0),
                            stop=False,
                        )
                    mmr(
                        psx[:, 0:1],
                        rowt[:, mo * 128 : (mo + 1) * 128],
                        selr[:, b, 512:513],
                        start=False,
                        stop=True,
                    )
                    gateT = smA.tile([128, 513], BF, tag="gateT", bufs=2, name="gateT")
                    act.activation(gateT[:, 0:512], ps, AF.Sigmoid)
                    act.activation(gateT[:, 512:513], psx[:, 0:1], AF.Sigmoid)
                    vec.scalar_tensor_tensor(
                        out=fusedT[:, mo, :],
                        in0=soT[:, b, mo, :],
                        scalar=bioc[:, mo, b : b + 1],
                        in1=gateT,
                        op0=ALU.subtract,
                        op1=ALU.mult,
                    )
                    vec.tensor_tensor(
                        out=fusedT[:, mo, :],
                        in0=soT[:, b, mo, :],
                        in1=fusedT[:, mo, :],
                        op=ALU.subtract,
                    )
                return f

            def proj_mo(b, mo, pp=None, gbufs=2):
                def f():
                    po_ = pp or ppB
                    wp = wref["wp"]
                    fusedT = wref[("fusedT", b)]
                    ps = po_.tile([128, 512], F32, tag="pgate", bufs=gbufs)
                    psx = po_.tile([128, 8], F32, tag="psm2", bufs=2)
                    for k in range(KC):
                        mmr(
                            ps,
                            wp[:, k, mo * 128 : (mo + 1) * 128],
                            fusedT[:, k, 0:512],
                            start=(k == 0),
                            stop=(k == KC - 1),
                        )
                        mmr(
                            psx[:, 0:1],
                            wp[:, k, mo * 128 : (mo + 1) * 128],
                            fusedT[:, k, 512:513],
                            start=(k == 0),
                            stop=(k == KC - 1),
                        )
                    outT = smA.tile([128, 513], F32, tag="outT", bufs=2, name="oT")
                    vec.tensor_scalar_add(outT[:, 0:512], ps, bp[:, mo : mo + 1])
                    vec.tensor_scalar_add(
                        outT[:, 512:513], psx[:, 0:1], bp[:, mo : mo + 1]
                    )
                    sync.dma_start(
                        out=out.ap()[b, mo * 128 : (mo + 1) * 128, :], in_=outT
                    )
                return f

            def gate_fuse(b):
                def f():
                    wref[("fusedT", b)] = smA.tile(
                        [128, KC, 513], BF, tag="fusedT", bufs=2, name="fusedT"
                    )
                return f

            def pump2():
                if drains:
                    drains.pop(0)()

            TW = [128, 128, 128, 128, 102]
            for b in range(BPC):
                for j in range(KC):
                    mq = j
                    ets = {}
                    psq = ppB.tile([128, 128], F32, tag="psm2", bufs=2)
                    pcq = psq[:65, 10:12]
                    dp = smB.tile([2, 513], F32, tag="dpair", bufs=2, name="dpair")
                    vec.memset(psq[:, 0:10], 0.0)
                    pcxs = [
                        ppB.tile([128, 512], F32, tag="pcx", name=f"pcx{_p}")
                        for _p in range(2)
                    ]
                    ets = {}
                    for t in range(6):
                        if t < 5:
                            o0, tw = TOK0[t], TW[t]
                            for par in range(2):
                                kblk = (6 if par == 0 else 12) + j
                                pss = ppB.tile(
                                    [128, 512], F32, tag="pss", bufs=2, name="pss"
                                )
                                mmr(
                                    pss[:tw, :],
                                    qkT[:, b, kblk, o0 : o0 + tw],
                                    qkT[:, b, mq, 0:512],
                                    start=True,
                                    stop=True,
                                )
                                mmr(
                                    psq[:tw, 5 * par + t : 5 * par + t + 1],
                                    qkT[:, b, kblk, o0 : o0 + tw],
                                    qkT[:, b, mq, 512:513],
                                    start=True,
                                    stop=True,
                                )
                                et = smB.tile(
                                    [128, 512], BF, tag="expT", bufs=4, name="expT"
                                )
                                act.activation(et[:tw], pss[:tw], AF.Exp, scale=ISC)
                                ets[(par, t)] = et
                        if t >= 1:
                            tp = TW[t - 1]
                            for par in range(2):
                                h = 2 * j + par
                                mmr(
                                    pcxs[par],
                                    vn[:tp, b, t - 1, 65 * h : 65 * h + 128],
                                    ets.pop((par, t - 1))[:tp],
                                    start=(t == 1),
                                    stop=(t == 5),
                                )
                    etq = smB.tile([128, 10], BF, tag="etq", bufs=2, name="etq")
                    act.activation(etq, psq[:, 0:10], AF.Exp, scale=ISC)
                    for par in range(2):
                        h = 2 * j + par
                        for t in range(5):
                            mmr(
                                pcq[:, par : par + 1],
                                vn[: TW[t], b, t, 65 * h : 65 * h + 65],
                                etq[: TW[t], 5 * par + t : 5 * par + t + 1],
                                start=(t == 0),
                                stop=(t == 4),
                            )
                    for par in range(2):
                        h = 2 * j + par
                        po = 64 * par
                        pcx = pcxs[par]
                        if par == 0:
                            vec.tensor_copy(out=dp[0:1, 0:512], in_=pcx[64:65, :])
                            vec.tensor_copy(
                                out=dp[0:1, 512:513], in_=pcq[64:65, 0:1]
                            )
                        else:
                            dstg = smB.tile(
                                [65, 513], F32, tag="dstg", bufs=2, name="dstg"
                            )
                            vec.tensor_copy(
                                out=dstg[64:65, 0:512], in_=pcx[64:65, :]
                            )
                            vec.tensor_copy(
                                out=dstg[64:65, 512:513], in_=pcq[64:65, 1:2]
                            )
                            sync.dma_start(out=dp[1:2], in_=dstg[64:65, :])
                        vec.tensor_copy(
                            out=soT[po : po + 64, b, mq, 0:512], in_=pcx[0:64]
                        )
                        vec.tensor_copy(
                            out=soT[po : po + 64, b, mq, 512:513],
                            in_=pcq[0:64, par : par + 1],
                        )
                    # per-pair normalization (overlaps the next pair's SA)
                    rp = smB.tile([2, 513], F32, tag="rpair", bufs=2, name="rpair")
                    vec.reciprocal(rp, dp)
                    rec1 = smB.tile([1, 513], F32, tag="rec0", bufs=2, name="rec1")
                    sync.dma_start(out=rec1, in_=rp[1:2])
                    for par in range(2):
                        po = 64 * par
                        rbc = smB.tile([128, 513], F32, tag="rbc", bufs=2, name="rbc")
                        gp.partition_broadcast(rbc, rp[0:1] if par == 0 else rec1)
                        vec.tensor_tensor(
                            out=soT[po : po + 64, b, mq, :],
                            in0=soT[po : po + 64, b, mq, :],
                            in1=rbc[po : po + 64],
                            op=ALU.mult,
                        )
                    pump2()
                    pump2()
                    pump2()

                if b == 0:
                    drains.append(gate_fuse(0))
                    drains.extend(gate_mo(0, mo) for mo in range(KC))
                    drains.extend(proj_mo(0, mo) for mo in range(KC))
            gate_fuse(1)()
            for mo in range(KC):
                gate_mo(1, mo)()
                pump2()
            for mo in range(KC):
                proj_mo(1, mo)()
                pump2()
            while drains:
                pump2()
            _sa_stack.close()


# ====================== host side ======================


def stage_inputs(inputs):
    """Pure layout/dtype staging of the full inputs into 8 per-core in_maps."""
    f = np.float32
    bf = NPBF
    x = np.asarray(inputs["x"], f)
    bio = np.asarray(inputs["bio_embed"], f)
    conf = np.asarray(inputs["confidence"], f)
    bco = np.asarray(inputs["base_coords"], f)
    offs = np.asarray(inputs["offsets"], f)

    W = {k: np.asarray(v, f) for k, v in inputs.items()}
    qkv_w = W["qkv_w"]
    qkv_b = W["qkv_b"]
    wv = qkv_w[2 * C :]
    w_vT = np.zeros((C, VW), f)
    b_vw = np.zeros((1, VW), f)
    for h in range(H):
        w_vT[:, 65 * h : 65 * h + 64] = wv[64 * h : 64 * h + 64].T
        b_vw[0, 65 * h : 65 * h + 64] = qkv_b[2 * C + 64 * h : 2 * C + 64 * h + 64]
        b_vw[0, 65 * h + 64] = 1.0

    ident = np.eye(128, dtype=f)
    gz, gy, gx = np.meshgrid(np.arange(G), np.arange(G), np.arange(G), indexing="ij")
    ngrid = -np.stack([gx.ravel(), gy.ravel(), gz.ravel()], axis=1).astype(f)
    splitT = np.zeros((2, 128), f)
    splitT[0, :64] = 1.0
    splitT[1, 64:] = 1.0

    bf_names = {
        "w_qkT", "w_vT", "w_caqT", "w_cak", "w_cavT", "w_bioqT", "w_caoT",
        "w_dbaqT", "w_dbakvT", "w_dbaoT", "w_spT", "w_g1T", "w_g2T", "w_g3T",
        "w_pT", "selr", "ident", "splitT", "hsplit", "b_vw", "b_g",
    }

    shared = {
        "w_qkT": qkv_w[: 2 * C].T,
        "w_vT": w_vT,
        "w_caqT": W["ca_in_w"][:C].T,
        "w_cak": W["ca_in_w"][C : 2 * C],
        "w_cavT": W["ca_in_w"][2 * C :].T,
        "w_bioqT": W["bio_query_w"].T,
        "w_caoT": W["ca_out_w"].T,
        "w_dbaqT": W["dba_in_w"][:C].T,
        "w_dbakvT": W["dba_in_w"][C:].T,
        "w_dbaoT": W["dba_out_w"].T,
        "w_spT": W["sp_w"].T,
        "w_g1T": W["gate_w"][:, :C].T,
        "w_g2T": W["gate_w"][:, C : 2 * C].T,
        "w_g3T": W["gate_w"][:, 2 * C :].T,
        "w_pT": W["proj_w"].T,
        "b_qk": qkv_b[: 2 * C],
        "b_vw": b_vw,
        "b_bioq": W["bio_query_b"],
        "b_caq": W["ca_in_b"][:C],
        "b_cav": W["ca_in_b"][2 * C :],
        "b_cao": W["ca_out_b"],
        "b_dbaq": W["dba_in_b"][:C],
        "b_dbav": W["dba_in_b"][2 * C :],
        "b_dbao": W["dba_out_b"],
        "b_sp": W["sp_b"],
        "b_g": np.broadcast_to(W["gate_b"].reshape(1, C), (BPC, C)),
        "selr": np.eye(BPC, dtype=f)[:, :, None] * np.ones((1, 1, 513), f),
        "b_p": W["proj_b"],
        "ident": ident,
        "ngrid": ngrid,
        "splitT": splitT,
        "hsplit": splitT.T,
    }
    shared["bcoordsT"] = np.broadcast_to(bco.T[:, None, :], (3, BPC, M)).reshape(1, 3, BPC * M)
    shared = {
        k: np.ascontiguousarray(v, bf if k in bf_names else f)
        for k, v in shared.items()
    }

    in_maps = []
    for c in range(NCORES):
        sl = slice(c * BPC, (c + 1) * BPC)
        m = dict(shared)
        m["xn"] = np.ascontiguousarray(x[sl], bf)
        m["xT"] = np.ascontiguousarray(x[sl].transpose(0, 2, 1), bf)
        m["bioT"] = np.ascontiguousarray(bio[sl].T, bf)
        m["confb"] = np.ascontiguousarray(
            np.broadcast_to(conf[sl].reshape(1, BPC), (128, BPC))
        )
        m["confT"] = np.ascontiguousarray(conf[sl].reshape(BPC, 1))
        m["offsT"] = np.ascontiguousarray(
            offs[sl].transpose(2, 0, 1).reshape(1, 3, BPC * M)
        )
        in_maps.append(m)
    return in_maps


_CACHE = {}


def get_nc():
    if "nc" not in _CACHE:
        nc = bacc.Bacc("TRN2", target_bir_lowering=False, debug=False)
        build(nc)
        _CACHE["nc"] = nc
    return _CACHE["nc"]


def _ensure_ntff_hook():
    """The agent image's antenv lacks axon_hooks; shim it so trace=True can
    reach the libaxon NTFF profiler (profiling only, test-harness use)."""
    import types

    try:
        import antenv.axon_hooks  # noqa: F401

        return
    except ImportError:
        pass
    mod = types.ModuleType("antenv.axon_hooks")
    state = {"h": None}
    mod.set_axon_ntff_profile_hook = lambda h: state.__setitem__("h", h)
    mod.get_axon_ntff_profile_hook = lambda: state["h"]
    sys.modules["antenv.axon_hooks"] = mod
    import antenv

    antenv.axon_hooks = mod
    try:
        from trn_agent_boot.trn_boot import _ntff_profile_via_ctypes

        hook = _ntff_profile_via_ctypes("/opt/axon/libaxon_pjrt.so")
        if hook is not None:
            mod.set_axon_ntff_profile_hook(hook)
    except Exception:
        pass


def kernel(**inputs):
    trace = bool(int(os.environ.get("KERNEL_TRACE", "0")))
    if trace:
        _ensure_ntff_hook()
    nc = get_nc()
    in_maps = stage_inputs(inputs)
    res = bass_utils.run_bass_kernel_spmd(
        nc,
        in_maps,
        core_ids=list(range(NCORES)),
        trace=trace,
    )
    _CACHE["last_result"] = res
    outT = np.stack([res.results[c]["outT"] for c in range(NCORES)])
    out = outT.reshape(B, C, N).transpose(0, 2, 1)
    return np.ascontiguousarray(out, dtype=np.float32)
